# revision 63
# baseline (speedup 1.0000x reference)
"""Evidence-constrained self-attention on 8 TRN2 NeuronCores.

Sharding: heads across cores (2 heads/core, all 4 batches); attention is fully
local per (batch, head); context vectors are resharded with two on-chip
AllToAlls (one per local head, each overlapping remaining attention) so the
output projection is sequence-sharded (each core produces 1024 rows).

Per-core pipeline (all big operands bf16, f32 PSUM accumulation):
  1. QKV projections from host-transposed x producing Q^T/K^T [128, 8192] in
     SBUF; V PE-transposed to [k, dk] layout with an appended ones-column so
     the softmax denominator rides the PV matmul as PSUM row 64. Projection
     chunks are emitted interleaved with attention pieces (shared PSUM slots,
     retained xt tiles).
  2. Attention per (b, h) pair in two q-half passes (one 2-bank ctx tile live
     at a time): S^T = K_blk @ Q^T so softmax needs no transposes, causal
     block skipping (~45% work saved), exp on ACT with fused 1/sqrt(dk)
     scale, evidence+causal mask as a 0/1 bf16 multiply, PV accumulation in
     PSUM. Pieces are software-pipelined (QK of piece i+1 before PV of i).
  3. AllToAll of ctx^T chunks with f32 denominators bitcast into two bf16
     rows per chunk; normalize via reciprocal + PE-broadcast matmul; output
     projection; out^T [1024, 1024] written per core.

Workarounds for this container's toolchain: every instruction is limited to
one semaphore wait (_split_multi_waits hoists extras onto NoOps), and
collective-dependent loads use the gpsimd DMA path to avoid head-of-line
blocking the sync-engine DMA queues.
"""

import numpy as np
import ml_dtypes


def _split_multi_waits(nc, max_waits: int = 1) -> int:
    """This container's walrus build allows at most ONE semaphore wait per
    instruction; Tile attaches several (notably on the kernel-tail Drain).
    Hoist all but the last wait onto single-wait NoOps inserted before the
    instruction on the same engine — semantically identical."""
    import concourse.mybir as mybir

    n_split = 0
    ctr = 0
    for f in nc.m.functions:
        stack = list(f.blocks)
        while stack:
            blk = stack.pop()
            insts = blk.instructions
            out = []
            changed = False
            for ins in insts:
                si = ins.sync_info
                if si is not None and len(si.on_wait) > max_waits:
                    waits = list(si.on_wait)
                    for w in waits[:-max_waits]:
                        nop = mybir.InstNoOp(
                            name=f"{ins.name}_wsplit{ctr}", ins=[], outs=[]
                        )
                        ctr += 1
                        nop.engine = ins.engine
                        nop.sync_info = mybir.SyncInfo(on_wait=[w], on_update=[])
                        out.append(nop)
                    si.on_wait = waits[-max_waits:]
                    changed = True
                    n_split += 1
                out.append(ins)
            if changed:
                blk.instructions = out
    return n_split

B, S, D = 4, 2048, 1024
H, DK = 16, 64
N_CORES = 8
R = B * S  # 8192 flattened rows
HPC = H // N_CORES  # heads per core = 2
DL = HPC * DK  # d_local = 128
QSH = R // N_CORES  # q rows per core after reshard = 1024
N_KB = S // 128  # 16 k-blocks per pair
N_RC = R // 512  # 16 row-chunks for projections
N_DC = D // 128  # 8 contraction chunks

# packed causal mask: block kb keeps only cols q >= 128*kb; all 16 blocks
# resident in SBUF (4.45MB), loaded in two DMAs at kernel start
MOFF = [0] * (N_KB + 1)
for _kb in range(N_KB):
    MOFF[_kb + 1] = MOFF[_kb] + (S - 128 * _kb)
MW = MOFF[N_KB]  # 17408 packed mask cols

# fp8e4m3 projection inputs: weights scaled up by 2^5 into fp8's normal
# range (sigma=0.02 is subnormal); compensated in the exp scale (Q'K' =
# 1024 QK) and in Wo (V' = 32 V)
WSCALE = 32.0

BF16 = ml_dtypes.bfloat16
F8 = ml_dtypes.float8_e4m3fn

_BUILD_CACHE = {}


def _build_nc(reps=1):
    import concourse.bass as bass
    import concourse.mybir as mybir
    from concourse import tile
    from contextlib import ExitStack

    dt = mybir.dt
    f32 = dt.float32
    f32r = dt.float32r
    bf16 = dt.bfloat16
    f8 = dt.float8e4
    DR = mybir.MatmulPerfMode.DoubleRow
    AF = mybir.ActivationFunctionType

    nc = bass.Bass()

    xT8 = nc.dram_tensor("xT8", [D, R], f8, kind="ExternalInput")
    xTb = nc.dram_tensor("xTb", [D, R], bf16, kind="ExternalInput")
    wqT = nc.dram_tensor("wqT", [D, DL], f8, kind="ExternalInput")
    wkT = nc.dram_tensor("wkT", [D, DL], f8, kind="ExternalInput")
    wvT = nc.dram_tensor("wvT", [D, DL], bf16, kind="ExternalInput")
    maskP = nc.dram_tensor("maskP", [128, MW], bf16, kind="ExternalInput")
    woT = nc.dram_tensor("woT", [128, N_DC * D], bf16, kind="ExternalInput")
    sel = nc.dram_tensor("sel", [2 * N_CORES, N_DC * 128], f32r, kind="ExternalInput")
    outT = nc.dram_tensor("outT", [D, QSH], f32, kind="ExternalOutput")

    with tile.TileContext(nc) as tc, ExitStack() as ctx:
        sb = ctx.enter_context(tc.tile_pool(name="sb", bufs=1))
        psum = ctx.enter_context(tc.tile_pool(name="psum", bufs=1, space="PSUM"))
        dram = ctx.enter_context(tc.tile_pool(name="dram", bufs=1, space="DRAM"))

        # ---- persistent SBUF tensors ----
        qt_sb = sb.tile([128, R], bf16, name="qt_sb")
        kt_sb = sb.tile([128, R], bf16, name="kt_sb")
        v_sb = sb.tile([128, N_CORES * N_KB * 65], bf16, name="v_sb")
        mask_sb = sb.tile([128, MW], bf16, name="mask_sb")
        wo_sb = sb.tile([128, N_DC * D], bf16, name="wo_sb")
        a2a_sb = sb.tile([128, N_DC * QSH], bf16, name="a2a_sb")
        wq_sb = sb.tile([128, D], f8, name="wq_sb")
        wk_sb = sb.tile([128, D], f8, name="wk_sb")
        wv_sb = sb.tile([128, D], bf16, name="wv_sb")
        sel_sb = sb.tile([2 * N_CORES, N_DC * 128], f32r, name="sel_sb")
        den_sb = sb.tile([2 * N_CORES, QSH], f32, name="den_sb")
        rden_sb = sb.tile([2 * N_CORES, QSH], f32r, name="rden_sb")

        # ---- DRAM bounce buffers for the collectives (split by local head
        # so the first AllToAll overlaps attention of the second head) ----
        a2aA_in = dram.tile([N_CORES * 66, QSH], bf16, name="a2aA_in")
        a2aA_out = dram.tile([N_CORES * 66, QSH], bf16, name="a2aA_out")
        a2aB_in = dram.tile([N_CORES * 66, QSH], bf16, name="a2aB_in")
        a2aB_out = dram.tile([N_CORES * 66, QSH], bf16, name="a2aB_out")
        a2a_ins = [a2aA_in, a2aB_in]
        xt_prefetched = {}
        proj_pre = set()

        def load_weights(eng=None):
            # weights: [D, 128] -> SBUF [128, (dc 128)] in one DMA each
            for wsb, wdr in ((wq_sb, wqT), (wk_sb, wkT), (wv_sb, wvT)):
                (eng or nc.sync).dma_start(
                    wsb[:].rearrange("p (c m) -> p c m", c=N_DC),
                    wdr[:].rearrange("(c p) m -> p c m", p=128),
                )

        def emit_body(_rep, deferred):
            if _rep == 0:
                load_weights()
            # ones column for V_aug (data cols overwritten below)
            nc.gpsimd.memset(v_sb[:], 1.0)

            # ---- phase 1: QKV projections (emitted in batch groups,
            # interleaved with that batch's head-0 attention) ----
            def load_xt8(rc, eng=None):
                r0 = rc * 512
                xt = sb.tile([128, N_DC * 512], f8, name="xt", tag="xt", bufs=4)
                (eng or nc.sync).dma_start(
                    xt[:].rearrange("p (c m) -> p c m", c=N_DC),
                    xT8[:].rearrange("(c p) m -> p c m", p=128)[:, :, r0 : r0 + 512],
                )
                return xt

            def load_xtb(rc, eng=None):
                r0 = rc * 512
                xtb = sb.tile([128, N_DC * 512], bf16, name="xtb", tag="xtb", bufs=3)
                (eng or nc.sync).dma_start(
                    xtb[:].rearrange("p (c m) -> p c m", c=N_DC),
                    xTb[:].rearrange("(c p) m -> p c m", p=128)[:, :, r0 : r0 + 512],
                )
                return xtb

            def rc_steps(rc):
                # two filler steps per projection chunk for finer interleave
                r0 = rc * 512
                st = {}

                def step_qk():
                    pre = xt_prefetched.pop(rc, {})
                    xt = pre.get("xt8") or load_xt8(rc)
                    xtb = pre.get("xtb") or load_xtb(rc)
                    st["xtb"] = xtb
                    xtv = xt[:].rearrange("p (c m) -> p c m", c=N_DC)
                    qk_ps = psum.tile([128, 1024], f32, name="qk_ps", tag="S", bufs=3)
                    for j in range(N_DC // 2):
                        first, last = j == 0, j == N_DC // 2 - 1
                        xsl = xtv[:, 2 * j : 2 * j + 2, :]
                        nc.tensor.matmul(
                            qk_ps[:, 0:512],
                            wq_sb[:, 2 * j * 128 : (2 * j + 2) * 128].rearrange(
                                "p (c m) -> p c m", c=2
                            ),
                            xsl, start=first, stop=last, perf_mode=DR,
                        )
                        nc.tensor.matmul(
                            qk_ps[:, 512:1024],
                            wk_sb[:, 2 * j * 128 : (2 * j + 2) * 128].rearrange(
                                "p (c m) -> p c m", c=2
                            ),
                            xsl, start=first, stop=last, perf_mode=DR,
                        )
                    # rc<4 runs at the body boundary where ACT is idle and
                    # DVE is the backlog; rc>=4 runs inside attention where
                    # ACT is exp-bound — split the evacuations accordingly
                    if rc < 4:
                        nc.scalar.activation(
                            qt_sb[:, r0 : r0 + 512], qk_ps[:, 0:512], AF.Identity
                        )
                        nc.scalar.activation(
                            kt_sb[:, r0 : r0 + 512], qk_ps[:, 512:1024], AF.Identity
                        )
                    else:
                        nc.vector.tensor_copy(qt_sb[:, r0 : r0 + 512], qk_ps[:, 0:512])
                        nc.vector.tensor_copy(
                            kt_sb[:, r0 : r0 + 512], qk_ps[:, 512:1024]
                        )

                def step_v():
                    # V with x as the stationary side: out [128 k-rows, 128 dk]
                    # lands directly in PV-lhsT layout — no PE transpose.
                    # bf16 (not fp8): V errors pass straight into the output
                    xtb = st["xtb"]
                    v_ps = psum.tile([128, 512], f32, name="v_ps", tag="S", bufs=3)
                    for rb4 in range(4):
                        out_sl = slice(rb4 * 128, rb4 * 128 + 128)
                        for dc_i in range(N_DC):
                            x0 = dc_i * 512 + rb4 * 128
                            nc.tensor.matmul(
                                v_ps[:, out_sl],
                                xtb[:, x0 : x0 + 128],
                                wv_sb[:, dc_i * 128 : (dc_i + 1) * 128],
                                start=dc_i == 0, stop=dc_i == N_DC - 1,
                            )
                    vv = v_sb[:].rearrange(
                        "p (b h k m) -> p b h k m", b=B, h=HPC, m=65
                    )
                    for rb4 in range(4):
                        rb = rc * 4 + rb4  # global 128-row block 0..63
                        b = rb // 16
                        kb = rb % 16
                        dst = vv[:, b : b + 1, :, kb : kb + 1, 0:64]
                        src = v_ps[:, rb4 * 128 : rb4 * 128 + 128].rearrange(
                            "p (h m) -> p h m", h=HPC
                        )
                        if rc < 4:
                            nc.scalar.activation(dst, src, AF.Identity)
                        else:
                            nc.vector.tensor_copy(dst, src)

                return [step_qk, step_v]

            def do_rc(rc):
                for s in rc_steps(rc):
                    s()

            # ---- phase 2: attention, head 0 pairs then head 1 pairs ----
            def do_pair(b, hl, fillers=()):
                fillers = list(fillers)
                p = b * HPC + hl
                row0 = b * S
                hs = slice(64 * hl, 64 * hl + 64)

                # two passes over q halves so only one 2-bank ctx tile is live:
                # pass 0: q in [0, 1024), kb 0..7; pass 1: q in [1024, 2048)
                # pieces: (kb, ph0, plen, evac_bank_or_None)
                passes = []
                for qlo, qhi, kbs in ((0, 1024, 8), (1024, 2048, 16)):
                    pieces = []
                    for kb in range(kbs):
                        ph0 = max(128 * kb, qlo)
                        plen = qhi - ph0
                        evac_c = None
                        if kb % 4 == 3 and qlo <= 512 * ((kb - 3) // 4) < qhi:
                            evac_c = (kb - 3) // 4
                        pieces.append((kb, ph0, plen, evac_c))
                    passes.append((qlo, qhi, pieces))

                EXP_SCALE = 0.125 / (WSCALE * WSCALE)

                def emit_qk(piece):
                    kb, ph0, plen, _ = piece
                    q0 = 128 * kb
                    kslice = slice(row0 + q0, row0 + q0 + 128)
                    s_ps = psum.tile([128, 1024], f32, name="s_ps", tag="S", bufs=3)
                    for sc0 in range(0, plen, 512):  # S-tile bank-aligned subs
                        slen = min(512, plen - sc0)
                        nc.tensor.matmul(
                            s_ps[:, sc0 : sc0 + slen],
                            kt_sb[hs, kslice],
                            qt_sb[hs, row0 + ph0 + sc0 : row0 + ph0 + sc0 + slen],
                            start=True,
                            stop=True,
                        )
                    return s_ps

                def emit_rest(piece, s_ps, ctx_ps, qlo):
                    kb, ph0, plen, evac_c = piece
                    pt = sb.tile([128, 1024], bf16, name="pt", tag="pt", bufs=4)
                    nc.scalar.activation(
                        pt[:, :plen], s_ps[:, :plen], AF.Exp, scale=EXP_SCALE
                    )
                    pm = sb.tile([128, 1024], bf16, name="pm", tag="pm", bufs=4)
                    m0 = MOFF[kb] + ph0 - 128 * kb
                    mtile = mask_sb[:, m0 : m0 + plen]
                    nc.vector.tensor_mul(pm[:, :plen], pt[:, :plen], mtile)
                    # PV accumulate; each matmul out must stay in one ctx bank
                    vbase = (p * N_KB + kb) * 65
                    g = ph0
                    while g < ph0 + plen:
                        glen = min(512 - (g % 512), ph0 + plen - g)
                        c = g // 512
                        last_kb = min(N_KB - 1, 4 * c + 3)
                        nc.tensor.matmul(
                            ctx_ps[:, g - qlo : g - qlo + glen],
                            v_sb[:, vbase : vbase + 65],
                            pm[:, g - ph0 : g - ph0 + glen],
                            start=(kb == 0),
                            stop=(kb == last_kb),
                        )
                        g += glen
                    if evac_c is not None:
                        c0 = 512 * evac_c
                        cc0 = c0 - qlo
                        ctxu = sb.tile([64, 512], bf16, name="ctxu", tag="ctxu", bufs=6)
                        nc.vector.tensor_copy(
                            ctxu[:], ctx_ps[0:64, cc0 : cc0 + 512]
                        )
                        dsb = sb.tile([65, 512], f32, name="dsb", tag="dsb", bufs=3)
                        nc.vector.tensor_copy(
                            dsb[64:65, :], ctx_ps[64:65, cc0 : cc0 + 512]
                        )
                        j = (row0 + c0) // QSH
                        t0 = (row0 + c0) % QSH
                        nc.sync.dma_start(
                            a2a_ins[hl][66 * j : 66 * j + 64, t0 : t0 + 512], ctxu[:]
                        )
                        dr = 66 * j + 64 + t0 // 512
                        nc.sync.dma_start(
                            a2a_ins[hl][dr : dr + 1, 0:1024],
                            dsb[64:65, :].bitcast(bf16),
                        )

                # interleave filler work (next batch's projection chunks)
                n_pieces = sum(len(pc) for _, _, pc in passes)
                stride = (
                    max(1, n_pieces // (len(fillers) + 1)) if fillers else 0
                )
                i = 0
                for pi, (qlo, qhi, pieces) in enumerate(passes):
                    ctx_ps = psum.tile(
                        [65, 1024], f32, name=f"ctx_{_rep}_{p}_{pi}",
                        uniquify=False, tag="ctx", bufs=1,
                    )
                    pending = []
                    for piece in pieces:
                        if fillers and i and i % stride == 0:
                            fillers.pop(0)()
                        i += 1
                        pending.append((piece, emit_qk(piece)))
                        if len(pending) > 2:
                            pc, ps = pending.pop(0)
                            emit_rest(pc, ps, ctx_ps, qlo)
                    for pc, ps in pending:
                        emit_rest(pc, ps, ctx_ps, qlo)
                for f in fillers:
                    f()

            # ---- phase 3 (previous rep's, deferred): loads, normalize,
            # output projection. Emitted as steps interleaved late into THIS
            # rep's head-0 pairs, by which point the previous rep's AllToAll-B
            # has landed — so these never head-of-line-block the in-order
            # engine queues. ----
            def phase3_steps():
                steps = []

                def s_loads():
                    # gpsimd (SWDGE) path: these wait on the collectives; on
                    # the sync/HWDGE queues they would HOL-block phase-2 DMAs
                    a2aA_v = a2aA_out[:].rearrange("(c p) q -> p c q", p=66)
                    a2aB_v = a2aB_out[:].rearrange("(c p) q -> p c q", p=66)
                    nc.gpsimd.dma_start(
                        a2a_sb[0:64, :].rearrange("p (c q) -> p c q", c=N_DC),
                        a2aA_v[0:64, :, :],
                    )
                    nc.gpsimd.dma_start(
                        a2a_sb[64:128, :].rearrange("p (c q) -> p c q", c=N_DC),
                        a2aB_v[0:64, :, :],
                    )
                    # den_sb rows: hl*8 + core; rows 64:66 of each chunk are
                    # the f32 denominator halves (bitcast)
                    denA_f = a2aA_out[:].bitcast(f32).rearrange("(c p) q -> c p q", p=66)
                    denB_f = a2aB_out[:].bitcast(f32).rearrange("(c p) q -> c p q", p=66)
                    nc.gpsimd.dma_start(
                        den_sb[0:8, :].rearrange("p (a q) -> p a q", a=2),
                        denA_f[:, 64:66, :],
                    )
                    nc.gpsimd.dma_start(
                        den_sb[8:16, :].rearrange("p (a q) -> p a q", a=2),
                        denB_f[:, 64:66, :],
                    )
                    with nc.allow_low_precision(
                        reason="f32r view of f32 reciprocal for PE bcast"
                    ):
                        nc.vector.reciprocal(rden_sb[:], den_sb[:])

                steps.append(s_loads)

                def s_norm(dc_i):
                    bc_ps = psum.tile([128, 1024], f32, name="bc_ps", tag="S", bufs=3)
                    for i in range(2):
                        nc.tensor.matmul(
                            bc_ps[:, i * 512 : (i + 1) * 512],
                            sel_sb[:, dc_i * 128 : (dc_i + 1) * 128],
                            rden_sb[:, i * 512 : (i + 1) * 512],
                            start=True,
                            stop=True,
                        )
                    dslice = slice(dc_i * QSH, (dc_i + 1) * QSH)
                    nc.vector.tensor_mul(a2a_sb[:, dslice], a2a_sb[:, dslice], bc_ps[:])

                def s_out(ec):
                    for qc in range(2):
                        op_ps = psum.tile([128, 1024], f32, name="op_ps", tag="S", bufs=3)
                        for dc_i in range(N_DC):
                            nc.tensor.matmul(
                                op_ps[:, 0:512],
                                wo_sb[:, dc_i * D + ec * 128 : dc_i * D + ec * 128 + 128],
                                a2a_sb[:, dc_i * QSH + qc * 512 : dc_i * QSH + qc * 512 + 512],
                                start=(dc_i == 0),
                                stop=(dc_i == N_DC - 1),
                            )
                        out_sb = sb.tile([128, 512], f32, name="out_sb", tag="out", bufs=3)
                        nc.scalar.activation(out_sb[:], op_ps[:, 0:512], AF.Identity)
                        nc.sync.dma_start(
                            outT[ec * 128 : (ec + 1) * 128, qc * 512 : (qc + 1) * 512],
                            out_sb[:],
                        )

                for dc_i in range(N_DC):
                    steps.append(lambda d=dc_i: s_norm(d))
                for ec in range(N_DC):
                    steps.append(lambda e=ec: s_out(e))
                return steps

            rg = [list(range(N_CORES))]
            # batch 0 projections first, then each head-0 pair interleaved with
            # the next batch's projection chunks
            # mask reload, one DMA per k-block: each block's WAR is only that
            # block's readers, which the previous body's last pair releases
            # incrementally — a single big DMA would wait for the very last
            # mask read and put its whole 12us transfer on the boundary
            for kb in range(N_KB):
                nc.sync.dma_start(
                    mask_sb[:, MOFF[kb] : MOFF[kb + 1]],
                    maskP[:, MOFF[kb] : MOFF[kb + 1]],
                )
            for rc in range(4):
                if rc in proj_pre:
                    proj_pre.discard(rc)  # emitted in the previous body's h1
                else:
                    do_rc(rc)
            # previous rep's phase 3 spreads over the last head-0 pair and
            # the head-1 pairs (its collective B has landed by then)
            nd = len(deferred)
            cut = [0, min(6, nd), min(12, nd), nd]
            pair_fills0 = [
                [s for rc in range(4, 8) for s in rc_steps(rc)],
                [s for rc in range(8, 12) for s in rc_steps(rc)],
                [s for rc in range(12, 16) for s in rc_steps(rc)],
                deferred[cut[0] : cut[1]],
            ]
            for b in range(B):
                do_pair(b, 0, fillers=pair_fills0[b])
            nc.gpsimd.collective_compute(
                "AllToAll", mybir.AluOpType.bypass, replica_groups=rg,
                ins=[a2aA_in.opt()], outs=[a2aA_out.opt()],
            )
            # next body's weights + first x chunks. Emitted HERE (before the
            # head-1 pairs) so they sit ahead of the h1 ctx-evacuation DMAs
            # on the SP queue — their WARs cleared during the head-0 phase,
            # so they issue immediately and transfer during h1 attention.
            load_weights()
            for rc in range(4):
                xt_prefetched.setdefault(rc, {})["xt8"] = load_xt8(rc)
            for rc in range(3):
                xt_prefetched[rc]["xtb"] = load_xtb(rc)
            pair_fills1 = [
                deferred[cut[1] : cut[2]],
                deferred[cut[2] :],
                [],
                [],
            ]
            for b in range(B):
                do_pair(b, 1, fillers=pair_fills1[b])
            nc.gpsimd.collective_compute(
                "AllToAll", mybir.AluOpType.bypass, replica_groups=rg,
                ins=[a2aB_in.opt()], outs=[a2aB_out.opt()],
            )
            # wo/sel for this body's deferred phase 3: by now the previous
            # phase 3 (their WAR) has completed, so no ACT-queue HOL block
            nc.scalar.dma_start(sel_sb[:], sel[:])
            nc.scalar.dma_start(wo_sb[:], woT[:])
            return phase3_steps()

        deferred = []
        for _rep in range(reps):
            deferred = emit_body(_rep, deferred)
        # final rep's phase 3 runs standalone at the end
        for s in deferred:
            s()

    _split_multi_waits(nc)
    return nc


def get_nc():
    if "nc" not in _BUILD_CACHE:
        _BUILD_CACHE["nc"] = _build_nc()
    return _BUILD_CACHE["nc"]


def make_in_maps(hidden_states, attention_mask, Wq, Wk, Wv, Wo):
    hs = np.asarray(hidden_states, dtype=np.float32)
    xT = hs.reshape(R, D).T
    xT8 = np.ascontiguousarray(xT.astype(F8))
    xTb = np.ascontiguousarray(xT.astype(BF16))
    mask01T = (np.asarray(attention_mask) == 0.0).T.astype(BF16)
    # packed causal layout: block kb keeps cols q >= 128*kb
    maskP = np.concatenate(
        [mask01T[kb * 128 : (kb + 1) * 128, 128 * kb :] for kb in range(N_KB)],
        axis=1,
    )
    maskP = np.ascontiguousarray(maskP)
    # woT[p, dc*D + e] = Wo[e, dc*128 + p]
    woT = np.ascontiguousarray(
        np.asarray(Wo, dtype=np.float32)
        .T.reshape(N_DC, 128, D)
        .transpose(1, 0, 2)
        .reshape(128, N_DC * D)
        .astype(BF16)
    )
    # den_sb row layout is hl*8 + core; a2a_sb block dc has head0 rows 0:64
    selm = np.zeros((2 * N_CORES, N_DC * 128), dtype=np.float32)
    for dc_i in range(N_DC):
        for pp in range(128):
            selm[(pp // 64) * N_CORES + dc_i, dc_i * 128 + pp] = 1.0
    in_maps = []
    for c in range(N_CORES):
        rows = slice(c * DL, (c + 1) * DL)
        in_maps.append(
            {
                "xT8": xT8,
                "xTb": xTb,
                "wqT": np.ascontiguousarray(
                    (np.asarray(Wq, np.float32)[rows] * WSCALE).T.astype(F8)
                ),
                "wkT": np.ascontiguousarray(
                    (np.asarray(Wk, np.float32)[rows] * WSCALE).T.astype(F8)
                ),
                "wvT": np.ascontiguousarray(
                    np.asarray(Wv, np.float32)[rows].T.astype(BF16)
                ),
                "maskP": maskP,
                "woT": woT,
                "sel": selm,
            }
        )
    return in_maps


def assemble_output(results):
    out = np.empty((R, D), dtype=np.float32)
    for c in range(N_CORES):
        out[c * QSH : (c + 1) * QSH] = results[c]["outT"].T
    return out.reshape(B, S, D)


def kernel(hidden_states, attention_mask, Wq, Wk, Wv, Wo):
    from concourse.bass_utils import run_bass_kernel_spmd

    nc = get_nc()
    in_maps = make_in_maps(hidden_states, attention_mask, Wq, Wk, Wv, Wo)
    res = run_bass_kernel_spmd(nc, in_maps, core_ids=list(range(N_CORES)))
    return assemble_output(res.results)



# revision 64
# speedup vs baseline: 1.0293x; 1.0293x over previous
"""Evidence-constrained self-attention on 8 TRN2 NeuronCores.

Sharding: heads across cores (2 heads/core, all 4 batches); attention is fully
local per (batch, head); context vectors are resharded with two on-chip
AllToAlls (one per local head, each overlapping remaining attention) so the
output projection is sequence-sharded (each core produces 1024 rows).

Per-core pipeline (f32 PSUM accumulation throughout):
  1. Q/K projections in fp8e4m3 with the DoubleRow perf mode (2x PE rate;
     host pre-quantizes x and 32-scaled weights, compensated in the exp
     scale); V projection in bf16 (V errors pass straight to the output)
     with x as the stationary operand so V lands directly in [k, dk] PV-lhsT
     layout — no PE transpose. An appended ones-column makes the softmax
     denominator ride the PV matmul as PSUM row 64.
  2. Attention per (b, h) pair in two q-half passes: S^T = K_blk @ Q^T so
     softmax needs no transposes, causal block skipping, exp on ACT with
     fused scale, evidence+causal mask as a 0/1 bf16 multiply on DVE (mask
     packed causally: only cols q >= 128*kb stored; fully SBUF-resident,
     reloaded per body in per-block DMAs so each block's WAR releases as the
     previous body's last pair finishes it). Pieces are software-pipelined.
  3. AllToAll of ctx^T chunks with f32 denominators bitcast into two bf16
     rows per chunk; normalize via reciprocal + PE-broadcast matmul; output
     projection; out^T [1024, 1024] written per core.

Steady-state (replicated-body) pipelining: each body's phase 3 is emitted as
filler steps inside the NEXT body's late pairs (all engine queues are
in-order, so B-collective-dependent work emitted inline would head-of-line
block them); the next body's weights and first x chunks are prefetched on
queue positions ahead of the h1 ctx-evacuation DMAs (DMA completion
semaphores are cumulative per HW queue).

Workarounds for this container's toolchain: every instruction is limited to
one semaphore wait (_split_multi_waits hoists extras onto NoOps), and
collective-dependent loads use the gpsimd DMA path to avoid head-of-line
blocking the sync-engine DMA queues.
"""

import numpy as np
import ml_dtypes


def _split_multi_waits(nc, max_waits: int = 1) -> int:
    """This container's walrus build allows at most ONE semaphore wait per
    instruction; Tile attaches several (notably on the kernel-tail Drain).
    Hoist all but the last wait onto single-wait NoOps inserted before the
    instruction on the same engine — semantically identical."""
    import concourse.mybir as mybir

    n_split = 0
    ctr = 0
    for f in nc.m.functions:
        stack = list(f.blocks)
        while stack:
            blk = stack.pop()
            insts = blk.instructions
            out = []
            changed = False
            for ins in insts:
                si = ins.sync_info
                if si is not None and len(si.on_wait) > max_waits:
                    waits = list(si.on_wait)
                    for w in waits[:-max_waits]:
                        nop = mybir.InstNoOp(
                            name=f"{ins.name}_wsplit{ctr}", ins=[], outs=[]
                        )
                        ctr += 1
                        nop.engine = ins.engine
                        nop.sync_info = mybir.SyncInfo(on_wait=[w], on_update=[])
                        out.append(nop)
                    si.on_wait = waits[-max_waits:]
                    changed = True
                    n_split += 1
                out.append(ins)
            if changed:
                blk.instructions = out
    return n_split

B, S, D = 4, 2048, 1024
H, DK = 16, 64
N_CORES = 8
R = B * S  # 8192 flattened rows
HPC = H // N_CORES  # heads per core = 2
DL = HPC * DK  # d_local = 128
QSH = R // N_CORES  # q rows per core after reshard = 1024
N_KB = S // 128  # 16 k-blocks per pair
N_RC = R // 512  # 16 row-chunks for projections
N_DC = D // 128  # 8 contraction chunks

# packed causal mask: block kb keeps only cols q >= 128*kb; all 16 blocks
# resident in SBUF (4.45MB), loaded in two DMAs at kernel start
MOFF = [0] * (N_KB + 1)
for _kb in range(N_KB):
    MOFF[_kb + 1] = MOFF[_kb] + (S - 128 * _kb)
MW = MOFF[N_KB]  # 17408 packed mask cols

# fp8e4m3 projection inputs: weights scaled up by 2^5 into fp8's normal
# range (sigma=0.02 is subnormal); compensated in the exp scale (Q'K' =
# 1024 QK) and in Wo (V' = 32 V)
WSCALE = 32.0

BF16 = ml_dtypes.bfloat16
F8 = ml_dtypes.float8_e4m3fn

_BUILD_CACHE = {}


def _build_nc(reps=1):
    import concourse.bass as bass
    import concourse.mybir as mybir
    from concourse import tile
    from contextlib import ExitStack

    dt = mybir.dt
    f32 = dt.float32
    f32r = dt.float32r
    bf16 = dt.bfloat16
    f8 = dt.float8e4
    DR = mybir.MatmulPerfMode.DoubleRow
    AF = mybir.ActivationFunctionType

    nc = bass.Bass()

    xT8 = nc.dram_tensor("xT8", [D, R], f8, kind="ExternalInput")
    xTb = nc.dram_tensor("xTb", [D, R], bf16, kind="ExternalInput")
    wqT = nc.dram_tensor("wqT", [D, DL], f8, kind="ExternalInput")
    wkT = nc.dram_tensor("wkT", [D, DL], f8, kind="ExternalInput")
    wvT = nc.dram_tensor("wvT", [D, DL], bf16, kind="ExternalInput")
    maskP = nc.dram_tensor("maskP", [128, MW], bf16, kind="ExternalInput")
    woT = nc.dram_tensor("woT", [128, N_DC * D], bf16, kind="ExternalInput")
    sel = nc.dram_tensor("sel", [2 * N_CORES, N_DC * 128], f32r, kind="ExternalInput")
    outT = nc.dram_tensor("outT", [D, QSH], f32, kind="ExternalOutput")

    with tile.TileContext(nc) as tc, ExitStack() as ctx:
        sb = ctx.enter_context(tc.tile_pool(name="sb", bufs=1))
        psum = ctx.enter_context(tc.tile_pool(name="psum", bufs=1, space="PSUM"))
        dram = ctx.enter_context(tc.tile_pool(name="dram", bufs=1, space="DRAM"))

        # ---- persistent SBUF tensors ----
        qt_sb = sb.tile([128, R], bf16, name="qt_sb")
        kt_sb = sb.tile([128, R], bf16, name="kt_sb")
        v_sb = sb.tile([128, N_CORES * N_KB * 65], bf16, name="v_sb")
        mask_sb = sb.tile([128, MW], bf16, name="mask_sb")
        wo_sb = sb.tile([128, N_DC * D], bf16, name="wo_sb")
        a2a_sb = sb.tile([128, N_DC * QSH], bf16, name="a2a_sb")
        wq_sb = sb.tile([128, D], f8, name="wq_sb")
        wk_sb = sb.tile([128, D], f8, name="wk_sb")
        wv_sb = sb.tile([128, D], bf16, name="wv_sb")
        sel_sb = sb.tile([2 * N_CORES, N_DC * 128], f32r, name="sel_sb")
        den_sb = sb.tile([2 * N_CORES, QSH], f32, name="den_sb")
        rden_sb = sb.tile([2 * N_CORES, QSH], f32r, name="rden_sb")

        # ---- DRAM bounce buffers for the collectives (split by local head
        # so the first AllToAll overlaps attention of the second head) ----
        a2aA_in = dram.tile([N_CORES * 66, QSH], bf16, name="a2aA_in")
        a2aA_out = dram.tile([N_CORES * 66, QSH], bf16, name="a2aA_out")
        a2aB_in = dram.tile([N_CORES * 66, QSH], bf16, name="a2aB_in")
        a2aB_out = dram.tile([N_CORES * 66, QSH], bf16, name="a2aB_out")
        a2a_ins = [a2aA_in, a2aB_in]
        xt_prefetched = {}
        proj_pre = set()

        def load_weights(eng=None):
            # weights: [D, 128] -> SBUF [128, (dc 128)] in one DMA each
            for wsb, wdr in ((wq_sb, wqT), (wk_sb, wkT), (wv_sb, wvT)):
                (eng or nc.sync).dma_start(
                    wsb[:].rearrange("p (c m) -> p c m", c=N_DC),
                    wdr[:].rearrange("(c p) m -> p c m", p=128),
                )

        def emit_body(_rep, deferred):
            if _rep == 0:
                load_weights()
            # ones column for V_aug (data cols overwritten below)
            nc.gpsimd.memset(v_sb[:], 1.0)

            # ---- phase 1: QKV projections (emitted in batch groups,
            # interleaved with that batch's head-0 attention) ----
            def load_xt8(rc, eng=None):
                r0 = rc * 512
                xt = sb.tile([128, N_DC * 512], f8, name="xt", tag="xt", bufs=4)
                (eng or nc.sync).dma_start(
                    xt[:].rearrange("p (c m) -> p c m", c=N_DC),
                    xT8[:].rearrange("(c p) m -> p c m", p=128)[:, :, r0 : r0 + 512],
                )
                return xt

            def load_xtb(rc, eng=None):
                r0 = rc * 512
                xtb = sb.tile([128, N_DC * 512], bf16, name="xtb", tag="xtb", bufs=3)
                (eng or nc.sync).dma_start(
                    xtb[:].rearrange("p (c m) -> p c m", c=N_DC),
                    xTb[:].rearrange("(c p) m -> p c m", p=128)[:, :, r0 : r0 + 512],
                )
                return xtb

            def rc_steps(rc):
                # two filler steps per projection chunk for finer interleave
                r0 = rc * 512
                st = {}

                def step_qk():
                    pre = xt_prefetched.pop(rc, {})
                    xt = pre.get("xt8") or load_xt8(rc)
                    xtb = pre.get("xtb") or load_xtb(rc)
                    st["xtb"] = xtb
                    xtv = xt[:].rearrange("p (c m) -> p c m", c=N_DC)
                    qk_ps = psum.tile([128, 1024], f32, name="qk_ps", tag="S", bufs=3)
                    for j in range(N_DC // 2):
                        first, last = j == 0, j == N_DC // 2 - 1
                        xsl = xtv[:, 2 * j : 2 * j + 2, :]
                        nc.tensor.matmul(
                            qk_ps[:, 0:512],
                            wq_sb[:, 2 * j * 128 : (2 * j + 2) * 128].rearrange(
                                "p (c m) -> p c m", c=2
                            ),
                            xsl, start=first, stop=last, perf_mode=DR,
                        )
                        nc.tensor.matmul(
                            qk_ps[:, 512:1024],
                            wk_sb[:, 2 * j * 128 : (2 * j + 2) * 128].rearrange(
                                "p (c m) -> p c m", c=2
                            ),
                            xsl, start=first, stop=last, perf_mode=DR,
                        )
                    # rc<4 runs at the body boundary where ACT is idle and
                    # DVE is the backlog; rc>=4 runs inside attention where
                    # ACT is exp-bound — split the evacuations accordingly
                    if rc < 4:
                        nc.scalar.activation(
                            qt_sb[:, r0 : r0 + 512], qk_ps[:, 0:512], AF.Identity
                        )
                        nc.scalar.activation(
                            kt_sb[:, r0 : r0 + 512], qk_ps[:, 512:1024], AF.Identity
                        )
                    else:
                        nc.vector.tensor_copy(qt_sb[:, r0 : r0 + 512], qk_ps[:, 0:512])
                        nc.vector.tensor_copy(
                            kt_sb[:, r0 : r0 + 512], qk_ps[:, 512:1024]
                        )

                def step_v():
                    # V with x as the stationary side: out [128 k-rows, 128 dk]
                    # lands directly in PV-lhsT layout — no PE transpose.
                    # bf16 (not fp8): V errors pass straight into the output
                    xtb = st["xtb"]
                    v_ps = psum.tile([128, 512], f32, name="v_ps", tag="S", bufs=3)
                    for rb4 in range(4):
                        out_sl = slice(rb4 * 128, rb4 * 128 + 128)
                        for dc_i in range(N_DC):
                            x0 = dc_i * 512 + rb4 * 128
                            nc.tensor.matmul(
                                v_ps[:, out_sl],
                                xtb[:, x0 : x0 + 128],
                                wv_sb[:, dc_i * 128 : (dc_i + 1) * 128],
                                start=dc_i == 0, stop=dc_i == N_DC - 1,
                            )
                    vv = v_sb[:].rearrange(
                        "p (b h k m) -> p b h k m", b=B, h=HPC, m=65
                    )
                    for rb4 in range(4):
                        rb = rc * 4 + rb4  # global 128-row block 0..63
                        b = rb // 16
                        kb = rb % 16
                        dst = vv[:, b : b + 1, :, kb : kb + 1, 0:64]
                        src = v_ps[:, rb4 * 128 : rb4 * 128 + 128].rearrange(
                            "p (h m) -> p h m", h=HPC
                        )
                        if rc < 4:
                            nc.scalar.activation(dst, src, AF.Identity)
                        else:
                            nc.vector.tensor_copy(dst, src)

                return [step_qk, step_v]

            def do_rc(rc):
                for s in rc_steps(rc):
                    s()

            # ---- phase 2: attention, head 0 pairs then head 1 pairs ----
            def do_pair(b, hl, fillers=()):
                fillers = list(fillers)
                p = b * HPC + hl
                row0 = b * S
                hs = slice(64 * hl, 64 * hl + 64)

                # two passes over q halves so only one 2-bank ctx tile is live:
                # pass 0: q in [0, 1024), kb 0..7; pass 1: q in [1024, 2048)
                # pieces: (kb, ph0, plen, evac_bank_or_None)
                passes = []
                for qlo, qhi, kbs in ((0, 1024, 8), (1024, 2048, 16)):
                    pieces = []
                    for kb in range(kbs):
                        ph0 = max(128 * kb, qlo)
                        plen = qhi - ph0
                        evac_c = None
                        if kb % 4 == 3 and qlo <= 512 * ((kb - 3) // 4) < qhi:
                            evac_c = (kb - 3) // 4
                        pieces.append((kb, ph0, plen, evac_c))
                    passes.append((qlo, qhi, pieces))

                EXP_SCALE = 0.125 / (WSCALE * WSCALE)

                def emit_qk(piece):
                    kb, ph0, plen, _ = piece
                    q0 = 128 * kb
                    kslice = slice(row0 + q0, row0 + q0 + 128)
                    s_ps = psum.tile([128, 1024], f32, name="s_ps", tag="S", bufs=3)
                    for sc0 in range(0, plen, 512):  # S-tile bank-aligned subs
                        slen = min(512, plen - sc0)
                        nc.tensor.matmul(
                            s_ps[:, sc0 : sc0 + slen],
                            kt_sb[hs, kslice],
                            qt_sb[hs, row0 + ph0 + sc0 : row0 + ph0 + sc0 + slen],
                            start=True,
                            stop=True,
                        )
                    return s_ps

                def emit_rest(piece, s_ps, ctx_ps, qlo):
                    kb, ph0, plen, evac_c = piece
                    pt = sb.tile([128, 1024], bf16, name="pt", tag="pt", bufs=4)
                    nc.scalar.activation(
                        pt[:, :plen], s_ps[:, :plen], AF.Exp, scale=EXP_SCALE
                    )
                    pm = sb.tile([128, 1024], bf16, name="pm", tag="pm", bufs=4)
                    m0 = MOFF[kb] + ph0 - 128 * kb
                    mtile = mask_sb[:, m0 : m0 + plen]
                    nc.vector.tensor_mul(pm[:, :plen], pt[:, :plen], mtile)
                    # PV accumulate; each matmul out must stay in one ctx bank
                    vbase = (p * N_KB + kb) * 65
                    g = ph0
                    while g < ph0 + plen:
                        glen = min(512 - (g % 512), ph0 + plen - g)
                        c = g // 512
                        last_kb = min(N_KB - 1, 4 * c + 3)
                        nc.tensor.matmul(
                            ctx_ps[:, g - qlo : g - qlo + glen],
                            v_sb[:, vbase : vbase + 65],
                            pm[:, g - ph0 : g - ph0 + glen],
                            start=(kb == 0),
                            stop=(kb == last_kb),
                        )
                        g += glen
                    if evac_c is not None:
                        c0 = 512 * evac_c
                        cc0 = c0 - qlo
                        ctxu = sb.tile([64, 512], bf16, name="ctxu", tag="ctxu", bufs=6)
                        nc.vector.tensor_copy(
                            ctxu[:], ctx_ps[0:64, cc0 : cc0 + 512]
                        )
                        dsb = sb.tile([65, 512], f32, name="dsb", tag="dsb", bufs=3)
                        nc.vector.tensor_copy(
                            dsb[64:65, :], ctx_ps[64:65, cc0 : cc0 + 512]
                        )
                        j = (row0 + c0) // QSH
                        t0 = (row0 + c0) % QSH
                        nc.sync.dma_start(
                            a2a_ins[hl][66 * j : 66 * j + 64, t0 : t0 + 512], ctxu[:]
                        )
                        dr = 66 * j + 64 + t0 // 512
                        nc.sync.dma_start(
                            a2a_ins[hl][dr : dr + 1, 0:1024],
                            dsb[64:65, :].bitcast(bf16),
                        )

                # interleave filler work (next batch's projection chunks)
                n_pieces = sum(len(pc) for _, _, pc in passes)
                stride = (
                    max(1, n_pieces // (len(fillers) + 1)) if fillers else 0
                )
                i = 0
                for pi, (qlo, qhi, pieces) in enumerate(passes):
                    ctx_ps = psum.tile(
                        [65, 1024], f32, name=f"ctx_{_rep}_{p}_{pi}",
                        uniquify=False, tag="ctx", bufs=1,
                    )
                    pending = []
                    for piece in pieces:
                        if fillers and i and i % stride == 0:
                            fillers.pop(0)()
                        i += 1
                        pending.append((piece, emit_qk(piece)))
                        if len(pending) > 2:
                            pc, ps = pending.pop(0)
                            emit_rest(pc, ps, ctx_ps, qlo)
                    for pc, ps in pending:
                        emit_rest(pc, ps, ctx_ps, qlo)
                for f in fillers:
                    f()

            # ---- phase 3 (previous rep's, deferred): loads, normalize,
            # output projection. Emitted as steps interleaved late into THIS
            # rep's head-0 pairs, by which point the previous rep's AllToAll-B
            # has landed — so these never head-of-line-block the in-order
            # engine queues. ----
            def phase3_steps():
                steps = []

                def s_loads():
                    # gpsimd (SWDGE) path: these wait on the collectives; on
                    # the sync/HWDGE queues they would HOL-block phase-2 DMAs
                    a2aA_v = a2aA_out[:].rearrange("(c p) q -> p c q", p=66)
                    a2aB_v = a2aB_out[:].rearrange("(c p) q -> p c q", p=66)
                    nc.gpsimd.dma_start(
                        a2a_sb[0:64, :].rearrange("p (c q) -> p c q", c=N_DC),
                        a2aA_v[0:64, :, :],
                    )
                    nc.gpsimd.dma_start(
                        a2a_sb[64:128, :].rearrange("p (c q) -> p c q", c=N_DC),
                        a2aB_v[0:64, :, :],
                    )
                    # den_sb rows: hl*8 + core; rows 64:66 of each chunk are
                    # the f32 denominator halves (bitcast)
                    denA_f = a2aA_out[:].bitcast(f32).rearrange("(c p) q -> c p q", p=66)
                    denB_f = a2aB_out[:].bitcast(f32).rearrange("(c p) q -> c p q", p=66)
                    nc.gpsimd.dma_start(
                        den_sb[0:8, :].rearrange("p (a q) -> p a q", a=2),
                        denA_f[:, 64:66, :],
                    )
                    nc.gpsimd.dma_start(
                        den_sb[8:16, :].rearrange("p (a q) -> p a q", a=2),
                        denB_f[:, 64:66, :],
                    )
                    with nc.allow_low_precision(
                        reason="f32r view of f32 reciprocal for PE bcast"
                    ):
                        nc.vector.reciprocal(rden_sb[:], den_sb[:])

                steps.append(s_loads)

                def s_norm(dc_i):
                    bc_ps = psum.tile([128, 1024], f32, name="bc_ps", tag="S", bufs=3)
                    for i in range(2):
                        nc.tensor.matmul(
                            bc_ps[:, i * 512 : (i + 1) * 512],
                            sel_sb[:, dc_i * 128 : (dc_i + 1) * 128],
                            rden_sb[:, i * 512 : (i + 1) * 512],
                            start=True,
                            stop=True,
                        )
                    dslice = slice(dc_i * QSH, (dc_i + 1) * QSH)
                    nc.vector.tensor_mul(a2a_sb[:, dslice], a2a_sb[:, dslice], bc_ps[:])

                def s_out(ec):
                    for qc in range(2):
                        op_ps = psum.tile([128, 1024], f32, name="op_ps", tag="S", bufs=3)
                        for dc_i in range(N_DC):
                            nc.tensor.matmul(
                                op_ps[:, 0:512],
                                wo_sb[:, dc_i * D + ec * 128 : dc_i * D + ec * 128 + 128],
                                a2a_sb[:, dc_i * QSH + qc * 512 : dc_i * QSH + qc * 512 + 512],
                                start=(dc_i == 0),
                                stop=(dc_i == N_DC - 1),
                            )
                        out_sb = sb.tile([128, 512], f32, name="out_sb", tag="out", bufs=3)
                        nc.scalar.activation(out_sb[:], op_ps[:, 0:512], AF.Identity)
                        nc.sync.dma_start(
                            outT[ec * 128 : (ec + 1) * 128, qc * 512 : (qc + 1) * 512],
                            out_sb[:],
                        )

                for dc_i in range(N_DC):
                    steps.append(lambda d=dc_i: s_norm(d))
                for ec in range(N_DC):
                    steps.append(lambda e=ec: s_out(e))
                return steps

            rg = [list(range(N_CORES))]
            # batch 0 projections first, then each head-0 pair interleaved with
            # the next batch's projection chunks
            # mask reload, one DMA per k-block: each block's WAR is only that
            # block's readers, which the previous body's last pair releases
            # incrementally — a single big DMA would wait for the very last
            # mask read and put its whole 12us transfer on the boundary
            for kb in range(N_KB):
                nc.sync.dma_start(
                    mask_sb[:, MOFF[kb] : MOFF[kb + 1]],
                    maskP[:, MOFF[kb] : MOFF[kb + 1]],
                )
            for rc in range(4):
                if rc in proj_pre:
                    proj_pre.discard(rc)  # emitted in the previous body's h1
                else:
                    do_rc(rc)
            # previous rep's phase 3 spreads over the last head-0 pair and
            # the head-1 pairs (its collective B has landed by then)
            nd = len(deferred)
            cut = [0, min(6, nd), min(12, nd), nd]
            pair_fills0 = [
                [s for rc in range(4, 8) for s in rc_steps(rc)],
                [s for rc in range(8, 12) for s in rc_steps(rc)],
                [s for rc in range(12, 16) for s in rc_steps(rc)],
                deferred[cut[0] : cut[1]],
            ]
            for b in range(B):
                do_pair(b, 0, fillers=pair_fills0[b])
            nc.gpsimd.collective_compute(
                "AllToAll", mybir.AluOpType.bypass, replica_groups=rg,
                ins=[a2aA_in.opt()], outs=[a2aA_out.opt()],
            )
            # next body's weights + first x chunks. Emitted HERE (before the
            # head-1 pairs) so they sit ahead of the h1 ctx-evacuation DMAs
            # on the SP queue — their WARs cleared during the head-0 phase,
            # so they issue immediately and transfer during h1 attention.
            load_weights()
            for rc in range(4):
                xt_prefetched.setdefault(rc, {})["xt8"] = load_xt8(rc)
            for rc in range(3):
                xt_prefetched[rc]["xtb"] = load_xtb(rc)
            pair_fills1 = [
                deferred[cut[1] : cut[2]],
                deferred[cut[2] :],
                [],
                [],
            ]
            for b in range(B):
                do_pair(b, 1, fillers=pair_fills1[b])
            nc.gpsimd.collective_compute(
                "AllToAll", mybir.AluOpType.bypass, replica_groups=rg,
                ins=[a2aB_in.opt()], outs=[a2aB_out.opt()],
            )
            # wo/sel for this body's deferred phase 3: by now the previous
            # phase 3 (their WAR) has completed, so no ACT-queue HOL block
            nc.scalar.dma_start(sel_sb[:], sel[:])
            nc.scalar.dma_start(wo_sb[:], woT[:])
            return phase3_steps()

        deferred = []
        for _rep in range(reps):
            deferred = emit_body(_rep, deferred)
        # final rep's phase 3 runs standalone at the end
        for s in deferred:
            s()

    _split_multi_waits(nc)
    return nc


def get_nc():
    if "nc" not in _BUILD_CACHE:
        _BUILD_CACHE["nc"] = _build_nc()
    return _BUILD_CACHE["nc"]


def make_in_maps(hidden_states, attention_mask, Wq, Wk, Wv, Wo):
    hs = np.asarray(hidden_states, dtype=np.float32)
    xT = hs.reshape(R, D).T
    xT8 = np.ascontiguousarray(xT.astype(F8))
    xTb = np.ascontiguousarray(xT.astype(BF16))
    mask01T = (np.asarray(attention_mask) == 0.0).T.astype(BF16)
    # packed causal layout: block kb keeps cols q >= 128*kb
    maskP = np.concatenate(
        [mask01T[kb * 128 : (kb + 1) * 128, 128 * kb :] for kb in range(N_KB)],
        axis=1,
    )
    maskP = np.ascontiguousarray(maskP)
    # woT[p, dc*D + e] = Wo[e, dc*128 + p]
    woT = np.ascontiguousarray(
        np.asarray(Wo, dtype=np.float32)
        .T.reshape(N_DC, 128, D)
        .transpose(1, 0, 2)
        .reshape(128, N_DC * D)
        .astype(BF16)
    )
    # den_sb row layout is hl*8 + core; a2a_sb block dc has head0 rows 0:64
    selm = np.zeros((2 * N_CORES, N_DC * 128), dtype=np.float32)
    for dc_i in range(N_DC):
        for pp in range(128):
            selm[(pp // 64) * N_CORES + dc_i, dc_i * 128 + pp] = 1.0
    in_maps = []
    for c in range(N_CORES):
        rows = slice(c * DL, (c + 1) * DL)
        in_maps.append(
            {
                "xT8": xT8,
                "xTb": xTb,
                "wqT": np.ascontiguousarray(
                    (np.asarray(Wq, np.float32)[rows] * WSCALE).T.astype(F8)
                ),
                "wkT": np.ascontiguousarray(
                    (np.asarray(Wk, np.float32)[rows] * WSCALE).T.astype(F8)
                ),
                "wvT": np.ascontiguousarray(
                    np.asarray(Wv, np.float32)[rows].T.astype(BF16)
                ),
                "maskP": maskP,
                "woT": woT,
                "sel": selm,
            }
        )
    return in_maps


def assemble_output(results):
    out = np.empty((R, D), dtype=np.float32)
    for c in range(N_CORES):
        out[c * QSH : (c + 1) * QSH] = results[c]["outT"].T
    return out.reshape(B, S, D)


def kernel(hidden_states, attention_mask, Wq, Wk, Wv, Wo):
    from concourse.bass_utils import run_bass_kernel_spmd

    nc = get_nc()
    in_maps = make_in_maps(hidden_states, attention_mask, Wq, Wk, Wv, Wo)
    res = run_bass_kernel_spmd(nc, in_maps, core_ids=list(range(N_CORES)))
    return assemble_output(res.results)



# revision 77
# speedup vs baseline: 1.2452x; 1.2097x over previous
"""Evidence-constrained self-attention on 8 TRN2 NeuronCores.

Sharding: heads across cores (2 heads/core, all 4 batches); attention is fully
local per (batch, head); context vectors are resharded with two on-chip
AllToAlls (one per local head, each overlapping remaining attention) so the
output projection is sequence-sharded (each core produces 1024 rows).

Per-core pipeline (f32 PSUM accumulation throughout):
  1. Q/K projections in fp8e4m3 with the DoubleRow perf mode (2x PE rate;
     host pre-quantizes x and 32-scaled weights, compensated in the exp
     scale); V projection in bf16 (V errors pass straight to the output)
     with x as the stationary operand so V lands directly in [k, dk] PV-lhsT
     layout — no PE transpose. An appended ones-column makes the softmax
     denominator ride the PV matmul as PSUM row 64.
  2. Attention per (b, h) pair in two q-half passes: S^T = K_blk @ Q^T so
     softmax needs no transposes, causal block skipping, exp on ACT with
     fused scale, evidence+causal mask as a 0/1 bf16 multiply on DVE (mask
     packed causally: only cols q >= 128*kb stored; fully SBUF-resident,
     reloaded per body in per-block DMAs so each block's WAR releases as the
     previous body's last pair finishes it). Pieces are software-pipelined.
  3. AllToAll of ctx^T chunks with f32 denominators bitcast into two bf16
     rows per chunk; normalize via reciprocal + PE-broadcast matmul; output
     projection; out^T [1024, 1024] written per core.

Steady-state (replicated-body) pipelining: each body's phase 3 is emitted as
filler steps inside the NEXT body's late pairs (all engine queues are
in-order, so B-collective-dependent work emitted inline would head-of-line
block them); the next body's weights and first x chunks are prefetched on
queue positions ahead of the h1 ctx-evacuation DMAs (DMA completion
semaphores are cumulative per HW queue).

Workarounds for this container's toolchain: every instruction is limited to
one semaphore wait (_split_multi_waits hoists extras onto NoOps), and
collective-dependent loads use the gpsimd DMA path to avoid head-of-line
blocking the sync-engine DMA queues.
"""

import numpy as np
import ml_dtypes


def _split_multi_waits(nc, max_waits: int = 1) -> int:
    """This container's walrus build allows at most ONE semaphore wait per
    instruction; Tile attaches several (notably on the kernel-tail Drain).
    Hoist all but the last wait onto single-wait NoOps inserted before the
    instruction on the same engine — semantically identical."""
    import concourse.mybir as mybir

    n_split = 0
    ctr = 0
    for f in nc.m.functions:
        stack = list(f.blocks)
        while stack:
            blk = stack.pop()
            insts = blk.instructions
            out = []
            changed = False
            for ins in insts:
                si = ins.sync_info
                if si is not None and len(si.on_wait) > max_waits:
                    waits = list(si.on_wait)
                    for w in waits[:-max_waits]:
                        nop = mybir.InstNoOp(
                            name=f"{ins.name}_wsplit{ctr}", ins=[], outs=[]
                        )
                        ctr += 1
                        nop.engine = ins.engine
                        nop.sync_info = mybir.SyncInfo(on_wait=[w], on_update=[])
                        out.append(nop)
                    si.on_wait = waits[-max_waits:]
                    changed = True
                    n_split += 1
                out.append(ins)
            if changed:
                blk.instructions = out
    return n_split

B, S, D = 4, 2048, 1024
H, DK = 16, 64
N_CORES = 8
R = B * S  # 8192 flattened rows
HPC = H // N_CORES  # heads per core = 2
DL = HPC * DK  # d_local = 128
QSH = R // N_CORES  # q rows per core after reshard = 1024
N_KB = S // 128  # 16 k-blocks per pair
N_RC = R // 512  # 16 row-chunks for projections
N_DC = D // 128  # 8 contraction chunks

# packed causal mask: block kb keeps only cols q >= 128*kb; all 16 blocks
# resident in SBUF (4.45MB), loaded in two DMAs at kernel start
MOFF = [0] * (N_KB + 1)
for _kb in range(N_KB):
    MOFF[_kb + 1] = MOFF[_kb] + (S - 128 * _kb)
MW = MOFF[N_KB]  # 17408 packed mask cols

# fp8e4m3 projection inputs: weights scaled up by 2^5 into fp8's normal
# range (sigma=0.02 is subnormal); compensated in the exp scale (Q'K' =
# 1024 QK) and in Wo (V' = 32 V)
WSCALE = 32.0

BF16 = ml_dtypes.bfloat16
F8 = ml_dtypes.float8_e4m3fn

_BUILD_CACHE = {}


def _build_nc(reps=1):
    import concourse.bass as bass
    import concourse.mybir as mybir
    from concourse import tile
    from contextlib import ExitStack

    dt = mybir.dt
    f32 = dt.float32
    f32r = dt.float32r
    bf16 = dt.bfloat16
    f8 = dt.float8e4
    DR = mybir.MatmulPerfMode.DoubleRow
    AF = mybir.ActivationFunctionType

    nc = bass.Bass()

    xT8 = nc.dram_tensor("xT8", [D, R], f8, kind="ExternalInput")
    xTb = nc.dram_tensor("xTb", [D, R], bf16, kind="ExternalInput")
    wqT = nc.dram_tensor("wqT", [D, DL], f8, kind="ExternalInput")
    wkT = nc.dram_tensor("wkT", [D, DL], f8, kind="ExternalInput")
    wvT = nc.dram_tensor("wvT", [D, DL], bf16, kind="ExternalInput")
    maskP = nc.dram_tensor("maskP", [128, MW], bf16, kind="ExternalInput")
    woT = nc.dram_tensor("woT", [128, N_DC * D], bf16, kind="ExternalInput")
    sel = nc.dram_tensor("sel", [2 * N_CORES, N_DC * 128], f32r, kind="ExternalInput")
    outT = nc.dram_tensor("outT", [D, QSH], f32, kind="ExternalOutput")

    with tile.TileContext(nc) as tc, ExitStack() as ctx:
        sb = ctx.enter_context(tc.tile_pool(name="sb", bufs=1))
        psum = ctx.enter_context(tc.tile_pool(name="psum", bufs=1, space="PSUM"))
        dram = ctx.enter_context(tc.tile_pool(name="dram", bufs=1, space="DRAM"))

        # ---- persistent SBUF tensors ----
        qt_sb = sb.tile([128, R], bf16, name="qt_sb")
        kt_sb = sb.tile([128, R], bf16, name="kt_sb")
        v_sb = sb.tile([128, N_CORES * N_KB * 65], bf16, name="v_sb")
        mask_sb = sb.tile([128, MW], bf16, name="mask_sb")
        wo_sb = sb.tile([128, N_DC * D], bf16, name="wo_sb")
        a2a_sb = sb.tile([128, N_DC * QSH], bf16, name="a2a_sb")
        wq_sb = sb.tile([128, D], f8, name="wq_sb")
        wk_sb = sb.tile([128, D], f8, name="wk_sb")
        wv_sb = sb.tile([128, D], bf16, name="wv_sb")
        sel_sb = sb.tile([2 * N_CORES, N_DC * 128], f32r, name="sel_sb")
        den_sb = sb.tile([2 * N_CORES, QSH], f32, name="den_sb")
        rden_sb = sb.tile([2 * N_CORES, QSH], f32r, name="rden_sb")

        # ---- DRAM bounce buffers for the collectives (split by local head
        # so the first AllToAll overlaps attention of the second head) ----
        a2aA_in = dram.tile([N_CORES * 66, QSH], bf16, name="a2aA_in")
        a2aA_out = dram.tile([N_CORES * 66, QSH], bf16, name="a2aA_out")
        a2aB_in = dram.tile([N_CORES * 66, QSH], bf16, name="a2aB_in")
        a2aB_out = dram.tile([N_CORES * 66, QSH], bf16, name="a2aB_out")
        a2a_ins = [a2aA_in, a2aB_in]
        xt_prefetched = {}
        proj_pre = set()

        def load_weights(eng=None):
            # weights: [D, 128] -> SBUF [128, (dc 128)] in one DMA each
            for wsb, wdr in ((wq_sb, wqT), (wk_sb, wkT), (wv_sb, wvT)):
                (eng or nc.sync).dma_start(
                    wsb[:].rearrange("p (c m) -> p c m", c=N_DC),
                    wdr[:].rearrange("(c p) m -> p c m", p=128),
                )

        def emit_body(_rep, deferred):
            if _rep == 0:
                load_weights()
            # ones column for V_aug (data cols overwritten below)
            nc.gpsimd.memset(v_sb[:], 1.0)

            # ---- phase 1: QKV projections (emitted in batch groups,
            # interleaved with that batch's head-0 attention) ----
            def load_xt8(rc, eng=None):
                r0 = rc * 512
                xt = sb.tile([128, N_DC * 512], f8, name="xt", tag="xt", bufs=4)
                (eng or nc.sync).dma_start(
                    xt[:].rearrange("p (c m) -> p c m", c=N_DC),
                    xT8[:].rearrange("(c p) m -> p c m", p=128)[:, :, r0 : r0 + 512],
                )
                return xt

            def load_xtb(rc, eng=None):
                r0 = rc * 512
                xtb = sb.tile([128, N_DC * 512], bf16, name="xtb", tag="xtb", bufs=3)
                (eng or nc.sync).dma_start(
                    xtb[:].rearrange("p (c m) -> p c m", c=N_DC),
                    xTb[:].rearrange("(c p) m -> p c m", p=128)[:, :, r0 : r0 + 512],
                )
                return xtb

            def rc_steps(rc):
                # two filler steps per projection chunk for finer interleave
                r0 = rc * 512
                st = {}

                def step_qk():
                    pre = xt_prefetched.pop(rc, {})
                    xt = pre.get("xt8") or load_xt8(rc)
                    xtb = pre.get("xtb") or load_xtb(rc)
                    st["xtb"] = xtb
                    xtv = xt[:].rearrange("p (c m) -> p c m", c=N_DC)
                    qk_ps = psum.tile([128, 1024], f32, name="qk_ps", tag="S", bufs=3)
                    for j in range(N_DC // 2):
                        first, last = j == 0, j == N_DC // 2 - 1
                        xsl = xtv[:, 2 * j : 2 * j + 2, :]
                        nc.tensor.matmul(
                            qk_ps[:, 0:512],
                            wq_sb[:, 2 * j * 128 : (2 * j + 2) * 128].rearrange(
                                "p (c m) -> p c m", c=2
                            ),
                            xsl, start=first, stop=last, perf_mode=DR,
                        )
                        nc.tensor.matmul(
                            qk_ps[:, 512:1024],
                            wk_sb[:, 2 * j * 128 : (2 * j + 2) * 128].rearrange(
                                "p (c m) -> p c m", c=2
                            ),
                            xsl, start=first, stop=last, perf_mode=DR,
                        )
                    # rc<4 runs at the body boundary: split each evacuation
                    # across BOTH engines so neither tail serializes the
                    # first attention pair; rc>=4 runs inside attention
                    # where ACT is exp-bound, so those go to DVE
                    if rc < 4:
                        nc.scalar.activation(
                            qt_sb[:, r0 : r0 + 512], qk_ps[:, 0:512], AF.Identity
                        )
                        nc.vector.tensor_copy(
                            kt_sb[:, r0 : r0 + 512], qk_ps[:, 512:1024]
                        )
                    else:
                        nc.vector.tensor_copy(qt_sb[:, r0 : r0 + 512], qk_ps[:, 0:512])
                        nc.vector.tensor_copy(
                            kt_sb[:, r0 : r0 + 512], qk_ps[:, 512:1024]
                        )

                def step_v():
                    # V with x as the stationary side: out [128 k-rows, 128 dk]
                    # lands directly in PV-lhsT layout — no PE transpose.
                    # bf16 (not fp8): V errors pass straight into the output
                    xtb = st["xtb"]
                    v_ps = psum.tile([128, 512], f32, name="v_ps", tag="S", bufs=3)
                    for rb4 in range(4):
                        out_sl = slice(rb4 * 128, rb4 * 128 + 128)
                        for dc_i in range(N_DC):
                            x0 = dc_i * 512 + rb4 * 128
                            nc.tensor.matmul(
                                v_ps[:, out_sl],
                                xtb[:, x0 : x0 + 128],
                                wv_sb[:, dc_i * 128 : (dc_i + 1) * 128],
                                start=dc_i == 0, stop=dc_i == N_DC - 1,
                            )
                    vv = v_sb[:].rearrange(
                        "p (b h k m) -> p b h k m", b=B, h=HPC, m=65
                    )
                    for rb4 in range(4):
                        rb = rc * 4 + rb4  # global 128-row block 0..63
                        b = rb // 16
                        kb = rb % 16
                        dst = vv[:, b : b + 1, :, kb : kb + 1, 0:64]
                        src = v_ps[:, rb4 * 128 : rb4 * 128 + 128].rearrange(
                            "p (h m) -> p h m", h=HPC
                        )
                        if rc < 4:
                            nc.scalar.activation(dst, src, AF.Identity)
                        else:
                            nc.vector.tensor_copy(dst, src)

                return [step_qk, step_v]

            def do_rc(rc):
                for s in rc_steps(rc):
                    s()

            # ---- phase 2: attention, head 0 pairs then head 1 pairs ----
            def do_pair(b, hl, fillers=()):
                fillers = list(fillers)
                p = b * HPC + hl
                row0 = b * S
                hs = slice(64 * hl, 64 * hl + 64)

                # two passes over q halves so only one 2-bank ctx tile is live:
                # pass 0: q in [0, 1024), kb 0..7; pass 1: q in [1024, 2048)
                # pieces: (kb, ph0, plen, evac_bank_or_None)
                passes = []
                for qlo, qhi, kbs in ((0, 1024, 8), (1024, 2048, 16)):
                    pieces = []
                    for kb in range(kbs):
                        ph0 = max(128 * kb, qlo)
                        plen = qhi - ph0
                        evac_c = None
                        if kb % 4 == 3 and qlo <= 512 * ((kb - 3) // 4) < qhi:
                            evac_c = (kb - 3) // 4
                        pieces.append((kb, ph0, plen, evac_c))
                    passes.append((qlo, qhi, pieces))

                EXP_SCALE = 0.125 / (WSCALE * WSCALE)

                def emit_qk(piece):
                    kb, ph0, plen, _ = piece
                    q0 = 128 * kb
                    kslice = slice(row0 + q0, row0 + q0 + 128)
                    s_ps = psum.tile([128, 1024], f32, name="s_ps", tag="S", bufs=3)
                    for sc0 in range(0, plen, 512):  # S-tile bank-aligned subs
                        slen = min(512, plen - sc0)
                        nc.tensor.matmul(
                            s_ps[:, sc0 : sc0 + slen],
                            kt_sb[hs, kslice],
                            qt_sb[hs, row0 + ph0 + sc0 : row0 + ph0 + sc0 + slen],
                            start=True,
                            stop=True,
                        )
                    return s_ps

                def emit_rest(piece, s_ps, ctx_ps, qlo):
                    kb, ph0, plen, evac_c = piece
                    pt = sb.tile([128, 1024], bf16, name="pt", tag="pt", bufs=4)
                    nc.scalar.activation(
                        pt[:, :plen], s_ps[:, :plen], AF.Exp, scale=EXP_SCALE
                    )
                    pm = sb.tile([128, 1024], bf16, name="pm", tag="pm", bufs=4)
                    m0 = MOFF[kb] + ph0 - 128 * kb
                    mtile = mask_sb[:, m0 : m0 + plen]
                    nc.vector.tensor_mul(pm[:, :plen], pt[:, :plen], mtile)
                    # PV accumulate; each matmul out must stay in one ctx bank
                    vbase = (p * N_KB + kb) * 65
                    g = ph0
                    while g < ph0 + plen:
                        glen = min(512 - (g % 512), ph0 + plen - g)
                        c = g // 512
                        last_kb = min(N_KB - 1, 4 * c + 3)
                        nc.tensor.matmul(
                            ctx_ps[:, g - qlo : g - qlo + glen],
                            v_sb[:, vbase : vbase + 65],
                            pm[:, g - ph0 : g - ph0 + glen],
                            start=(kb == 0),
                            stop=(kb == last_kb),
                        )
                        g += glen
                    if evac_c is not None:
                        c0 = 512 * evac_c
                        cc0 = c0 - qlo
                        ctxu = sb.tile([64, 512], bf16, name="ctxu", tag="ctxu", bufs=6)
                        nc.vector.tensor_copy(
                            ctxu[:], ctx_ps[0:64, cc0 : cc0 + 512]
                        )
                        dsb = sb.tile([65, 512], f32, name="dsb", tag="dsb", bufs=3)
                        nc.vector.tensor_copy(
                            dsb[64:65, :], ctx_ps[64:65, cc0 : cc0 + 512]
                        )
                        j = (row0 + c0) // QSH
                        t0 = (row0 + c0) % QSH
                        nc.sync.dma_start(
                            a2a_ins[hl][66 * j : 66 * j + 64, t0 : t0 + 512], ctxu[:]
                        )
                        dr = 66 * j + 64 + t0 // 512
                        nc.sync.dma_start(
                            a2a_ins[hl][dr : dr + 1, 0:1024],
                            dsb[64:65, :].bitcast(bf16),
                        )

                # interleave filler work (next batch's projection chunks)
                n_pieces = sum(len(pc) for _, _, pc in passes)
                stride = (
                    max(1, n_pieces // (len(fillers) + 1)) if fillers else 0
                )
                i = 0
                for pi, (qlo, qhi, pieces) in enumerate(passes):
                    ctx_ps = psum.tile(
                        [65, 1024], f32, name=f"ctx_{_rep}_{p}_{pi}",
                        uniquify=False, tag="ctx", bufs=1,
                    )
                    pending = []
                    for piece in pieces:
                        if fillers and i and i % stride == 0:
                            fillers.pop(0)()
                        i += 1
                        pending.append((piece, emit_qk(piece)))
                        if len(pending) > 2:
                            pc, ps = pending.pop(0)
                            emit_rest(pc, ps, ctx_ps, qlo)
                    for pc, ps in pending:
                        emit_rest(pc, ps, ctx_ps, qlo)
                for f in fillers:
                    f()

            # ---- phase 3 (previous rep's, deferred): loads, normalize,
            # output projection. Emitted as steps interleaved late into THIS
            # rep's head-0 pairs, by which point the previous rep's AllToAll-B
            # has landed — so these never head-of-line-block the in-order
            # engine queues. ----
            def phase3_steps():
                steps = []

                def s_loads():
                    # gpsimd (SWDGE) path: these wait on the collectives; on
                    # the sync/HWDGE queues they would HOL-block phase-2 DMAs
                    a2aA_v = a2aA_out[:].rearrange("(c p) q -> p c q", p=66)
                    a2aB_v = a2aB_out[:].rearrange("(c p) q -> p c q", p=66)
                    nc.gpsimd.dma_start(
                        a2a_sb[0:64, :].rearrange("p (c q) -> p c q", c=N_DC),
                        a2aA_v[0:64, :, :],
                    )
                    nc.gpsimd.dma_start(
                        a2a_sb[64:128, :].rearrange("p (c q) -> p c q", c=N_DC),
                        a2aB_v[0:64, :, :],
                    )
                    # den_sb rows: hl*8 + core; rows 64:66 of each chunk are
                    # the f32 denominator halves (bitcast)
                    denA_f = a2aA_out[:].bitcast(f32).rearrange("(c p) q -> c p q", p=66)
                    denB_f = a2aB_out[:].bitcast(f32).rearrange("(c p) q -> c p q", p=66)
                    nc.gpsimd.dma_start(
                        den_sb[0:8, :].rearrange("p (a q) -> p a q", a=2),
                        denA_f[:, 64:66, :],
                    )
                    nc.gpsimd.dma_start(
                        den_sb[8:16, :].rearrange("p (a q) -> p a q", a=2),
                        denB_f[:, 64:66, :],
                    )
                    with nc.allow_low_precision(
                        reason="f32r view of f32 reciprocal for PE bcast"
                    ):
                        nc.vector.reciprocal(rden_sb[:], den_sb[:])

                steps.append(s_loads)

                def s_norm(dc_i):
                    bc_ps = psum.tile([128, 1024], f32, name="bc_ps", tag="S", bufs=3)
                    for i in range(2):
                        nc.tensor.matmul(
                            bc_ps[:, i * 512 : (i + 1) * 512],
                            sel_sb[:, dc_i * 128 : (dc_i + 1) * 128],
                            rden_sb[:, i * 512 : (i + 1) * 512],
                            start=True,
                            stop=True,
                        )
                    dslice = slice(dc_i * QSH, (dc_i + 1) * QSH)
                    nc.vector.tensor_mul(a2a_sb[:, dslice], a2a_sb[:, dslice], bc_ps[:])

                def s_out(ec, qcs=(0, 1)):
                    for qc in qcs:
                        op_ps = psum.tile([128, 1024], f32, name="op_ps", tag="S", bufs=3)
                        for dc_i in range(N_DC):
                            nc.tensor.matmul(
                                op_ps[:, 0:512],
                                wo_sb[:, dc_i * D + ec * 128 : dc_i * D + ec * 128 + 128],
                                a2a_sb[:, dc_i * QSH + qc * 512 : dc_i * QSH + qc * 512 + 512],
                                start=(dc_i == 0),
                                stop=(dc_i == N_DC - 1),
                            )
                        out_sb = sb.tile([128, 512], f32, name="out_sb", tag="out", bufs=3)
                        # alternate evac engine: these run inside exp-busy
                        # attention windows, so split the load ACT/DVE
                        if qc == 0:
                            nc.scalar.activation(out_sb[:], op_ps[:, 0:512], AF.Identity)
                        else:
                            nc.vector.tensor_copy(out_sb[:], op_ps[:, 0:512])
                        nc.sync.dma_start(
                            outT[ec * 128 : (ec + 1) * 128, qc * 512 : (qc + 1) * 512],
                            out_sb[:],
                        )

                for dc_i in range(N_DC):
                    steps.append(lambda d=dc_i: s_norm(d))
                for ec in range(N_DC):
                    for qc in range(2):
                        steps.append(lambda e=ec, q=qc: s_out(e, qcs=(q,)))
                return steps

            rg = [list(range(N_CORES))]
            # batch 0 projections first, then each head-0 pair interleaved with
            # the next batch's projection chunks
            # mask reload, one DMA per k-block: each block's WAR is only that
            # block's readers, which the previous body's last pair releases
            # incrementally — a single big DMA would wait for the very last
            # mask read and put its whole 12us transfer on the boundary
            for kb in range(N_KB):
                nc.sync.dma_start(
                    mask_sb[:, MOFF[kb] : MOFF[kb + 1]],
                    maskP[:, MOFF[kb] : MOFF[kb + 1]],
                )
            for rc in range(4):
                if rc in proj_pre:
                    proj_pre.discard(rc)  # emitted in the previous body's h1
                else:
                    do_rc(rc)
            # previous rep's phase 3 spreads over the last head-0 pair and
            # the head-1 pairs (its collective B has landed by then)
            nd = len(deferred)
            cut = [0, min(7, nd), min(13, nd), min(19, nd), nd]
            pair_fills0 = [
                [s for rc in range(4, 8) for s in rc_steps(rc)],
                [s for rc in range(8, 12) for s in rc_steps(rc)],
                [s for rc in range(12, 16) for s in rc_steps(rc)],
                deferred[cut[0] : cut[1]],
            ]
            for b in range(B):
                do_pair(b, 0, fillers=pair_fills0[b])
            nc.gpsimd.collective_compute(
                "AllToAll", mybir.AluOpType.bypass, replica_groups=rg,
                ins=[a2aA_in.opt()], outs=[a2aA_out.opt()],
            )
            # next body's weights + first x chunks. Emitted HERE (before the
            # head-1 pairs) so they sit ahead of the h1 ctx-evacuation DMAs
            # on the SP queue — their WARs cleared during the head-0 phase,
            # so they issue immediately and transfer during h1 attention.
            load_weights()
            for rc in range(4):
                xt_prefetched.setdefault(rc, {})["xt8"] = load_xt8(rc)
            for rc in range(3):
                xt_prefetched[rc]["xtb"] = load_xtb(rc)
            pair_fills1 = [
                deferred[cut[1] : cut[2]],
                deferred[cut[2] : cut[3]],
                deferred[cut[3] :],
                [],
            ]
            for b in range(B):
                do_pair(b, 1, fillers=pair_fills1[b])
            nc.gpsimd.collective_compute(
                "AllToAll", mybir.AluOpType.bypass, replica_groups=rg,
                ins=[a2aB_in.opt()], outs=[a2aB_out.opt()],
            )
            # wo/sel for this body's deferred phase 3: by now the previous
            # phase 3 (their WAR) has completed, so no ACT-queue HOL block
            nc.scalar.dma_start(sel_sb[:], sel[:])
            nc.scalar.dma_start(wo_sb[:], woT[:])
            return phase3_steps()

        deferred = []
        for _rep in range(reps):
            deferred = emit_body(_rep, deferred)
        # final rep's phase 3 runs standalone at the end
        for s in deferred:
            s()

    _split_multi_waits(nc)
    return nc


def get_nc():
    if "nc" not in _BUILD_CACHE:
        _BUILD_CACHE["nc"] = _build_nc()
    return _BUILD_CACHE["nc"]


def make_in_maps(hidden_states, attention_mask, Wq, Wk, Wv, Wo):
    hs = np.asarray(hidden_states, dtype=np.float32)
    xT = hs.reshape(R, D).T
    xT8 = np.ascontiguousarray(xT.astype(F8))
    xTb = np.ascontiguousarray(xT.astype(BF16))
    mask01T = (np.asarray(attention_mask) == 0.0).T.astype(BF16)
    # packed causal layout: block kb keeps cols q >= 128*kb
    maskP = np.concatenate(
        [mask01T[kb * 128 : (kb + 1) * 128, 128 * kb :] for kb in range(N_KB)],
        axis=1,
    )
    maskP = np.ascontiguousarray(maskP)
    # woT[p, dc*D + e] = Wo[e, dc*128 + p]
    woT = np.ascontiguousarray(
        np.asarray(Wo, dtype=np.float32)
        .T.reshape(N_DC, 128, D)
        .transpose(1, 0, 2)
        .reshape(128, N_DC * D)
        .astype(BF16)
    )
    # den_sb row layout is hl*8 + core; a2a_sb block dc has head0 rows 0:64
    selm = np.zeros((2 * N_CORES, N_DC * 128), dtype=np.float32)
    for dc_i in range(N_DC):
        for pp in range(128):
            selm[(pp // 64) * N_CORES + dc_i, dc_i * 128 + pp] = 1.0
    in_maps = []
    for c in range(N_CORES):
        rows = slice(c * DL, (c + 1) * DL)
        in_maps.append(
            {
                "xT8": xT8,
                "xTb": xTb,
                "wqT": np.ascontiguousarray(
                    (np.asarray(Wq, np.float32)[rows] * WSCALE).T.astype(F8)
                ),
                "wkT": np.ascontiguousarray(
                    (np.asarray(Wk, np.float32)[rows] * WSCALE).T.astype(F8)
                ),
                "wvT": np.ascontiguousarray(
                    np.asarray(Wv, np.float32)[rows].T.astype(BF16)
                ),
                "maskP": maskP,
                "woT": woT,
                "sel": selm,
            }
        )
    return in_maps


def assemble_output(results):
    out = np.empty((R, D), dtype=np.float32)
    for c in range(N_CORES):
        out[c * QSH : (c + 1) * QSH] = results[c]["outT"].T
    return out.reshape(B, S, D)


def kernel(hidden_states, attention_mask, Wq, Wk, Wv, Wo):
    from concourse.bass_utils import run_bass_kernel_spmd

    nc = get_nc()
    in_maps = make_in_maps(hidden_states, attention_mask, Wq, Wk, Wv, Wo)
    res = run_bass_kernel_spmd(nc, in_maps, core_ids=list(range(N_CORES)))
    return assemble_output(res.results)

